# revision 1
# baseline (speedup 1.0000x reference)
"""Trainium2 Bass kernel: sparse 7x7x7 stride-1 max-pool over a 64^3 voxel grid
(MinkowskiEngine semantics) + per-point MLP (1x1 conv -> ReLU -> 1x1 conv ->
sigmoid) * feats.

Strategy (8 NeuronCores, SPMD, no collectives):
  - Shard the dense grid along z: core k owns z in [8k, 8k+8), works on a
    14-plane z-slab (3-voxel halo each side, replicated halo build -> no
    cross-core exchange needed).
  - Stream over x-planes. Each dense (y,z)-plane slab is built on-device:
    DMA-memset a per-plane DRAM slab to -1e30, indirect-scatter the
    plane's occupied feats rows (host precomputes destination row ids),
    reload contiguously into SBUF ("scatter" mode; a pure indirect-gather
    "gather" mode is kept as an alternative, it loads the gpsimd engine
    more).
  - PE transposes each plane to [channel-part, (y,z)-free]; separable
    windowed max (7 = 4+4 overlap -> 3 tensor_max per axis) on DVE along
    z, then y (free dims), then x (across streamed planes). bf16
    throughout the pooling (max is order-preserving; only input rounding
    error).
  - Fused per-plane MLP on PE: h_T = relu(W1h.T @ pooled) (R on
    partitions), y2 = h_T_chunk.T @ W2 (vox on partitions -> natural
    row layout for free), sigmoid on ACT, plane written to a DRAM
    sig-slab.
  - Phase D: indirect row-gather of sig at owned points, multiply by
    exact fp32 feats rows on DVE, write sparse output rows; host
    scatters the 8 per-core row blocks back to [N, C].

Measured on the 8-core axon TRN2 fleet: HW exec ~0.95-1.0 ms, rel err
~4.6e-3 vs the fp32 reference (fp32 variant FULL_F32: 1.55 ms, 8e-7).
"""

from contextlib import ExitStack
from dataclasses import dataclass

import numpy as np

C = 256
R = 128
SENT = -1.0e30


@dataclass(frozen=True)
class Cfg:
    D: int = 64           # grid extent per axis
    ZS: int = 8           # owned z-planes per core
    NPTS: int = 100000    # total points
    OCT_PAD: int = 1792   # padded owned points per x-octant (mult of 128)
    ncores: int = 8
    dt_slab: str = "float32"   # dtype of gathered plane data (gather source)
    dt_pool: str = "float32"   # dtype of pooling intermediates
    dt_mm: str = "float32"     # dtype of matmul weights/activations in SBUF
    dt_sig: str = "float32"    # dtype of sigmoid slab
    plane_build: str = "gather"  # "gather" or "scatter"
    SCPAD: int = 512           # scatter rows per plane (mult of 128)

    @property
    def ZH(self):
        return self.ZS + 6

    @property
    def YZ(self):
        return self.D * self.ZH

    @property
    def T(self):
        return (self.YZ + 127) // 128

    @property
    def YZP(self):
        return self.T * 128

    @property
    def NX(self):
        return self.D

    @property
    def VOXH(self):
        return self.D * self.ZS      # owned voxels per x-plane

    @property
    def NV(self):
        return self.NPTS + 1

    @property
    def NP_PAD(self):
        return 8 * self.OCT_PAD

    @property
    def NT_D(self):
        return self.NP_PAD // 128

    @property
    def NT_O(self):
        return self.OCT_PAD // 128

    @property
    def PPO(self):
        return self.NX // 8


FULL = Cfg(
    dt_slab="bfloat16", dt_pool="bfloat16", dt_mm="bfloat16", dt_sig="bfloat16",
    plane_build="scatter",
)
FULL_F32 = Cfg()


def _np_dt(name):
    import ml_dtypes

    return {"float32": np.float32, "bfloat16": ml_dtypes.bfloat16}[name]


def build_nc(cfg: Cfg):
    """Build the (SPMD, per-core-identical) Bass program."""
    import concourse.bacc as bacc
    import concourse.bass as bass
    import concourse.tile as tile
    from concourse import mybir
    from concourse.masks import make_identity

    AF = mybir.ActivationFunctionType
    f32 = mybir.dt.float32
    i32 = mybir.dt.int32
    dts = getattr(mybir.dt, cfg.dt_slab)
    dtp = getattr(mybir.dt, cfg.dt_pool)
    dtm = getattr(mybir.dt, cfg.dt_mm)
    dtg = getattr(mybir.dt, cfg.dt_sig)

    D, ZS, ZH, T, YZ, YZP, NX = cfg.D, cfg.ZS, cfg.ZH, cfg.T, cfg.YZ, cfg.YZP, cfg.NX
    VOXH = cfg.VOXH
    VOX2 = 2 * VOXH
    NCH = VOXH // 128  # y2 chunks per plane

    nc = bacc.Bacc("TRN2", target_bir_lowering=False, debug=False,
                   enable_asserts=False, num_devices=cfg.ncores)

    featsp = nc.dram_tensor("featsp", [cfg.NP_PAD, C], f32, kind="ExternalInput").ap()
    if cfg.plane_build == "gather":
        featsg = nc.dram_tensor("featsg", [cfg.NV, C], dts, kind="ExternalInput").ap()
        gidx = nc.dram_tensor("gidx", [128, NX * T], i32, kind="ExternalInput").ap()
    else:
        SCT = cfg.SCPAD // 128
        featss = nc.dram_tensor(
            "featss", [NX * cfg.SCPAD, C], dts, kind="ExternalInput"
        ).ap()
        soff = nc.dram_tensor("soff", [128, NX * SCT], i32, kind="ExternalInput").ap()
    goff = nc.dram_tensor("goff", [128, cfg.NT_D], i32, kind="ExternalInput").ap()
    w1 = nc.dram_tensor("w1", [C, R], dtm, kind="ExternalInput").ap()
    w2 = nc.dram_tensor("w2", [R, C], dtm, kind="ExternalInput").ap()
    out = nc.dram_tensor("out", [cfg.NP_PAD, C], f32, kind="ExternalOutput").ap()

    with tile.TileContext(nc) as tc, ExitStack() as ctx:
        const = ctx.enter_context(tc.tile_pool(name="const", bufs=1))
        dram = ctx.enter_context(tc.tile_pool(name="dram", bufs=1, space="DRAM"))
        natp = ctx.enter_context(tc.tile_pool(name="natp", bufs=6))
        tpinp = ctx.enter_context(tc.tile_pool(name="tpinp", bufs=4, space="PSUM"))
        pp = ctx.enter_context(tc.tile_pool(name="pp", bufs=2))
        ztp = ctx.enter_context(tc.tile_pool(name="ztp", bufs=3))
        ytp = ctx.enter_context(tc.tile_pool(name="ytp", bufs=3))
        oyp = ctx.enter_context(tc.tile_pool(name="oyp", bufs=6))
        m2xp = ctx.enter_context(tc.tile_pool(name="m2xp", bufs=6))
        m4xp = ctx.enter_context(tc.tile_pool(name="m4xp", bufs=8))
        pxp = ctx.enter_context(tc.tile_pool(name="pxp", bufs=3))
        hpp = ctx.enter_context(tc.tile_pool(name="hpp", bufs=2, space="PSUM"))
        y2p = ctx.enter_context(tc.tile_pool(name="y2p", bufs=1, space="PSUM"))
        hsp = ctx.enter_context(tc.tile_pool(name="hsp", bufs=3))
        sgp = ctx.enter_context(tc.tile_pool(name="sgp", bufs=3))
        dp = ctx.enter_context(tc.tile_pool(name="dp", bufs=16))

        PPO = cfg.PPO
        outocts = [dram.tile([PPO * VOXH, C], dtg, name=f"oo{o}") for o in range(8)]

        # ---- constants
        ident = const.tile([128, 128], dts)
        make_identity(nc, ident[:])
        neg = const.tile([128, VOX2], dtp)
        nc.gpsimd.memset(neg[:], SENT)
        w1sb = const.tile([128, 2 * R], dtm)
        nc.sync.dma_start(
            w1sb[:].rearrange("p (h r) -> p h r", h=2),
            w1.rearrange("(h p) r -> p h r", p=128),
        )
        w2sb = const.tile([128, C], dtm)
        nc.sync.dma_start(w2sb[:], w2)
        if cfg.plane_build == "gather":
            gidx_sb = const.tile([128, NX * T], i32)
            nc.sync.dma_start(gidx_sb[:], gidx)
        else:
            SCT = cfg.SCPAD // 128
            soff_sb = const.tile([128, NX * SCT], i32)
            nc.sync.dma_start(soff_sb[:], soff)
            negnat = const.tile([128, T * C], dts)
            nc.gpsimd.memset(negnat[:], SENT)
            slabs = [dram.tile([YZP, C], dts, name=f"slab{x}") for x in range(NX)]
            negnat3 = negnat[:].rearrange("p (t c) -> p t c", t=T)

            def memset_slab(x):
                nc.scalar.dma_start(
                    slabs[x][:].rearrange("(t p) c -> p t c", p=128), negnat3
                )

            for x in range(6):
                memset_slab(x)
            scp = ctx.enter_context(tc.tile_pool(name="scp", bufs=16))
        goff_sb = const.tile([128, cfg.NT_D], i32)
        nc.sync.dma_start(goff_sb[:], goff)

        # persistent y-padded buffer; borders memset once
        ypad = const.tile([128, 2 * (D + 6) * ZS], dtp)
        ypv = v4_ypad = ypad[:].rearrange("p (h a b) -> p h a b", h=2, a=D + 6)
        nc.gpsimd.memset(ypv[:, :, 0:3, :], SENT)
        nc.gpsimd.memset(ypv[:, :, D + 3:D + 6, :], SENT)

        w1v = w1sb[:].rearrange("p (h r) -> p h r", h=2)

        def v4(ap, h, a, b):
            return ap.rearrange("p (h a b) -> p h a b", h=h, a=a)

        oy_t, m2x_t, m4x_t = {}, {}, {}

        def oy_at(j):
            return oy_t.get(j, neg)

        def m2x_at(j):
            return m2x_t.get(j, neg)

        for i in range(NX + 3):
            if i < NX:
                # ---- build natural plane [yz-part, C]
                nat = natp.tile([128, T * C], dts)
                if cfg.plane_build == "gather":
                    # indirect row gather straight into SBUF
                    for t in range(T):
                        nc.gpsimd.indirect_dma_start(
                            out=nat[:, t * C:(t + 1) * C],
                            out_offset=None,
                            in_=featsg,
                            in_offset=bass.IndirectOffsetOnAxis(
                                ap=gidx_sb[:, i * T + t: i * T + t + 1], axis=0
                            ),
                        )
                else:
                    # memset plane slab, indirect-scatter occupied rows, reload
                    slab = slabs[i]
                    slab3 = slab[:].rearrange("(t p) c -> p t c", p=128)
                    if i + 6 < NX:
                        memset_slab(i + 6)
                    for t in range(SCT):
                        col = i * SCT + t
                        fs = scp.tile([128, C], dts, tag="fs", name="fs")
                        nc.scalar.dma_start(
                            fs[:], featss[col * 128:(col + 1) * 128, :]
                        )
                        nc.gpsimd.indirect_dma_start(
                            out=slab[:],
                            out_offset=bass.IndirectOffsetOnAxis(
                                ap=soff_sb[:, col:col + 1], axis=0
                            ),
                            in_=fs[:],
                            in_offset=None,
                        )
                    nc.sync.dma_start(
                        nat[:].rearrange("p (t c) -> p t c", t=T), slab3
                    )
                # ---- PE transpose to [c-part, yz]
                tp0 = tpinp.tile([128, YZP], dts, tag="tp", name="tp0")
                tp1 = tpinp.tile([128, YZP], dts, tag="tp", name="tp1")
                for h, tp in ((0, tp0), (1, tp1)):
                    for t in range(T):
                        nc.tensor.transpose(
                            out=tp[:, t * 128:(t + 1) * 128],
                            in_=nat[:, t * C + h * 128: t * C + h * 128 + 128],
                            identity=ident[:],
                        )
                P = pp.tile([128, 2 * YZP], dtp)
                nc.scalar.activation(P[:, 0:YZP], tp0[:], AF.Copy)
                nc.scalar.activation(P[:, YZP:2 * YZP], tp1[:], AF.Copy)

                # ---- z-pass (window 7 over ZH -> ZS outputs)
                Pz = P[:].rearrange("p (h yz) -> p h yz", h=2)[:, :, 0:YZ] \
                    .rearrange("p h (y z) -> p h y z", z=ZH)
                m2z = ztp.tile([128, 2 * D * (ZH - 1)], dtp)
                m2zv = v4(m2z[:], 2, D, ZH - 1)
                nc.vector.tensor_max(m2zv, Pz[:, :, :, 0:ZH - 1], Pz[:, :, :, 1:ZH])
                m4z = ztp.tile([128, 2 * D * (ZH - 3)], dtp)
                m4zv = v4(m4z[:], 2, D, ZH - 3)
                nc.vector.tensor_max(m4zv, m2zv[:, :, :, 0:ZH - 3], m2zv[:, :, :, 2:ZH - 1])
                nc.vector.tensor_max(
                    ypv[:, :, 3:3 + D, :], m4zv[:, :, :, 0:ZS], m4zv[:, :, :, 3:3 + ZS]
                )

                # ---- y-pass
                m2y = ytp.tile([128, 2 * (D + 5) * ZS], dtp)
                m2yv = v4(m2y[:], 2, D + 5, ZS)
                nc.vector.tensor_max(m2yv, ypv[:, :, 0:D + 5, :], ypv[:, :, 1:D + 6, :])
                m4y = ytp.tile([128, 2 * (D + 3) * ZS], dtp)
                m4yv = v4(m4y[:], 2, D + 3, ZS)
                nc.vector.tensor_max(m4yv, m2yv[:, :, 0:D + 3, :], m2yv[:, :, 2:D + 5, :])
                oy = oyp.tile([128, VOX2], dtp)
                oyv = v4(oy[:], 2, D, ZS)
                nc.vector.tensor_max(oyv, m4yv[:, :, 0:D, :], m4yv[:, :, 3:D + 3, :])
                oy_t[i] = oy
            else:
                oy_t[i] = neg

            # ---- x-pass (streamed)
            j = i - 1
            if j >= NX:
                m2x_t[j] = neg
            else:
                m2x = m2xp.tile([128, VOX2], dtp)
                nc.vector.tensor_max(m2x[:], oy_at(j)[:], oy_at(j + 1)[:])
                m2x_t[j] = m2x
            j = i - 3
            if j >= NX:
                m4x_t[j] = neg
            else:
                a, b = m2x_at(j), m2x_at(j + 2)
                if a is neg and b is neg:
                    m4x_t[j] = neg
                else:
                    m4x = m4xp.tile([128, VOX2], dtp)
                    nc.vector.tensor_max(m4x[:], a[:], b[:])
                    m4x_t[j] = m4x
            k = i - 3
            if 0 <= k < NX:
                px = pxp.tile([128, VOX2], dtp)
                nc.vector.tensor_max(px[:], m4x_t.get(k - 3, neg)[:], m4x_t[k][:])

                # ---- MLP on plane k
                pxv = px[:].rearrange("p (h v) -> p h v", h=2)
                hp = hpp.tile([128, VOXH], f32, space="PSUM")
                for h in (0, 1):
                    nc.tensor.matmul(
                        hp[:], w1v[:, h, :], pxv[:, h, :], start=(h == 0), stop=(h == 1)
                    )
                hs = hsp.tile([128, VOXH], dtm)
                nc.scalar.activation(hs[:], hp[:], AF.Relu)
                y2 = y2p.tile([128, NCH * C], f32, space="PSUM")
                for jj in range(NCH):
                    nc.tensor.matmul(
                        y2[:, jj * C:(jj + 1) * C],
                        hs[:, jj * 128:(jj + 1) * 128],
                        w2sb[:],
                        start=True,
                        stop=True,
                    )
                sg = sgp.tile([128, NCH * C], dtg)
                nc.scalar.activation(sg[:], y2[:], AF.Sigmoid)
                dst = outocts[k // PPO][
                    (k % PPO) * VOXH:(k % PPO + 1) * VOXH, :
                ].rearrange("(t p) c -> p t c", p=128)
                nc.sync.dma_start(dst, sg[:].rearrange("p (t c) -> p t c", t=NCH))

        # ---- phase D: sparse gather + multiply
        for t in range(cfg.NT_D):
            sgrow = dp.tile([128, C], dtg)
            nc.gpsimd.indirect_dma_start(
                out=sgrow[:],
                out_offset=None,
                in_=outocts[t // cfg.NT_O][:],
                in_offset=bass.IndirectOffsetOnAxis(ap=goff_sb[:, t:t + 1], axis=0),
            )
            frow = dp.tile([128, C], f32)
            nc.sync.dma_start(frow[:], featsp[t * 128:(t + 1) * 128, :])
            orow = dp.tile([128, C], f32)
            nc.vector.tensor_mul(orow[:], sgrow[:], frow[:])
            nc.sync.dma_start(out[t * 128:(t + 1) * 128, :], orow[:])

    nc.compile()
    return nc


def host_prep(cfg: Cfg, feats, coords, W1, W2):
    """Shard/index-prep on host. Returns (in_maps, pid_pads)."""
    D, ZS, ZH, T, NX = cfg.D, cfg.ZS, cfg.ZH, cfg.T, cfg.NX
    dts_np = _np_dt(cfg.dt_slab)
    dtm_np = _np_dt(cfg.dt_mm)
    NPTS = cfg.NPTS

    ix = coords[:, 0].astype(np.int64)
    iy = coords[:, 1].astype(np.int64)
    iz = coords[:, 2].astype(np.int64)
    lin = (ix * D + iy) * D + iz
    inv = np.full(D * D * D, NPTS, np.int32)
    inv[lin] = np.arange(NPTS, dtype=np.int32)

    feats_ext = np.concatenate(
        [feats.astype(dts_np), np.full((1, C), SENT, dts_np)], axis=0
    )
    feats_ext = np.ascontiguousarray(feats_ext)
    w1h = np.ascontiguousarray(W1.astype(dtm_np))
    w2h = np.ascontiguousarray(W2.astype(dtm_np))

    xs = np.arange(D)[:, None, None]
    ys = np.arange(D)[None, :, None]

    in_maps, pid_pads = [], []
    for k in range(cfg.ncores):
        zlo = k * ZS - 3
        zs_ = zlo + np.arange(ZH)
        valid = (zs_ >= 0) & (zs_ < D)
        if cfg.plane_build == "gather":
            lin3 = (xs * D + ys) * D + np.clip(zs_, 0, D - 1)[None, None, :]
            g = np.where(valid[None, None, :], inv[lin3], NPTS).astype(np.int32)
            yzp = np.full((NX, T * 128), NPTS, np.int32)
            yzp[:, : D * ZH] = g.reshape(NX, D * ZH)
            gidx_sb = np.ascontiguousarray(
                yzp.reshape(NX, T, 128).transpose(2, 0, 1).reshape(128, NX * T)
            )
        else:
            SCPAD = cfg.SCPAD
            SCT = SCPAD // 128
            in_slab = (iz >= zlo) & (iz < zlo + ZH)
            featss = np.zeros((NX * SCPAD, C), dts_np)
            soff = np.zeros((NX, SCPAD), np.int32)
            for x in range(NX):
                sel = np.where(in_slab & (ix == x))[0]
                n = len(sel)
                assert n <= SCPAD, f"core {k} plane {x}: {n} > SCPAD"
                dest = (iy[sel] * ZH + (iz[sel] - zlo)).astype(np.int32)
                if n == 0:
                    rows = np.full((SCPAD, C), SENT, dts_np)
                    drows = np.zeros(SCPAD, np.int32)
                else:
                    rows = feats[sel].astype(dts_np)
                    rows = np.concatenate(
                        [rows, np.repeat(rows[-1:], SCPAD - n, axis=0)]
                    )
                    drows = np.concatenate(
                        [dest, np.full(SCPAD - n, dest[-1], np.int32)]
                    )
                featss[x * SCPAD:(x + 1) * SCPAD] = rows
                soff[x] = drows
            featss_k = np.ascontiguousarray(featss)
            soff_sb = np.ascontiguousarray(
                soff.reshape(NX, SCT, 128).transpose(2, 0, 1).reshape(128, NX * SCT)
            )

        own = (iz >= k * ZS) & (iz < (k + 1) * ZS)
        PPO = cfg.PPO
        parts = []
        for o in range(8):
            po = np.where(own & (ix // PPO == o))[0].astype(np.int64)
            npo = len(po)
            assert 0 < npo <= cfg.OCT_PAD, f"core {k} oct {o}: {npo}"
            parts.append(
                np.concatenate([po, np.full(cfg.OCT_PAD - npo, po[0], np.int64)])
            )
        pid_pad = np.concatenate(parts)
        gr = (
            ((ix[pid_pad] % PPO) * D + iy[pid_pad]) * ZS + (iz[pid_pad] - k * ZS)
        ).astype(np.int32)
        goff_sb = np.ascontiguousarray(gr.reshape(cfg.NT_D, 128).T)
        featsp = np.ascontiguousarray(feats[pid_pad].astype(np.float32))

        m = {"featsp": featsp, "goff": goff_sb, "w1": w1h, "w2": w2h}
        if cfg.plane_build == "gather":
            m["featsg"] = feats_ext
            m["gidx"] = gidx_sb
        else:
            m["featss"] = featss_k
            m["soff"] = soff_sb
        in_maps.append(m)
        pid_pads.append(pid_pad)
    return in_maps, pid_pads


_CACHE = {}


def _get_nc(cfg: Cfg):
    if cfg not in _CACHE:
        _CACHE[cfg] = build_nc(cfg)
    return _CACHE[cfg]


def kernel(feats, coords, W1, W2):
    from concourse.bass_utils import run_bass_kernel_spmd

    cfg = FULL
    nc = _get_nc(cfg)
    in_maps, pid_pads = host_prep(
        cfg,
        np.asarray(feats, np.float32),
        np.asarray(coords),
        np.asarray(W1, np.float32),
        np.asarray(W2, np.float32),
    )
    res = run_bass_kernel_spmd(nc, in_maps, core_ids=list(range(cfg.ncores)))
    out_full = np.empty((cfg.NPTS, C), np.float32)
    for k in range(cfg.ncores):
        out_full[pid_pads[k]] = res.results[k]["out"]
    return out_full



# revision 6
# speedup vs baseline: 1.1436x; 1.1436x over previous
"""Trainium2 Bass kernel: sparse 7x7x7 stride-1 max-pool over a 64^3 voxel grid
(MinkowskiEngine semantics) + per-point MLP (1x1 conv -> ReLU -> 1x1 conv ->
sigmoid) * feats.

Strategy (8 NeuronCores, SPMD, no collectives):
  - Shard the dense grid along z: core k owns z in [8k, 8k+8), and processes a
    14-plane z-slab (3-voxel halo each side; halo replicated on host -> no
    cross-core exchange).
  - The HOST builds the dense slab in the exact on-chip layout the kernel
    wants: per x-plane [128 part = ch%128, 2 ch-halves, 14 z, 72 y] with y
    pre-padded by 4 sentinel columns each side and empty voxels = -1e30.
    This is pure data marshalling (scatter + transpose + pad); all max/matmul
    arithmetic stays on device.
  - Device streams over the 64 x-planes: contiguous plane load, separable
    windowed max (7 = max of two 4-windows) on DVE along z then y (both in
    aligned 2x-mode layouts), then x across streamed planes; fused MLP on PE
    with the second matmul transposed (W2^T as weights) so sigmoid lands in
    [ch, vox] layout; dense multiply sg * plane-center on DVE; contiguous
    store of the dense product.
  - Host gathers the occupied voxels from the dense product planes (reverse
    marshalling) and casts to fp32.
"""

from contextlib import ExitStack
from dataclasses import dataclass

import numpy as np

C = 256
R = 128
SENT = -1.0e30


@dataclass(frozen=True)
class Cfg:
    D: int = 64           # grid extent per axis
    ZS: int = 8           # owned z-planes per core
    NPTS: int = 100000    # total points
    ncores: int = 8
    dt: str = "bfloat16"  # dtype of planes / pooling / matmul inputs
    PB: int = 2           # planes per build batch (z/y pass batching)

    @property
    def ZH(self):
        return self.ZS + 6

    @property
    def YP(self):
        return self.D + 8  # y padded to 72 (4 sentinel cols each side)

    @property
    def NX(self):
        return self.D

    @property
    def PLF(self):
        return 2 * self.ZH * self.YP  # free elems per partition per plane

    @property
    def VOX2(self):
        return 2 * self.ZS * self.D  # compact pooled free elems (2h * 8z * 64y)


FULL = Cfg()


def _np_dt(name):
    import ml_dtypes

    return {"bfloat16": ml_dtypes.bfloat16, "float16": np.float16}[name]


def build_nc(cfg: Cfg):
    """Build the (SPMD, per-core-identical) Bass program."""
    import concourse.bacc as bacc
    import concourse.tile as tile
    from concourse import mybir

    AF = mybir.ActivationFunctionType
    f32 = mybir.dt.float32
    dt = getattr(mybir.dt, cfg.dt)

    D, ZS, ZH, YP, NX, PB = cfg.D, cfg.ZS, cfg.ZH, cfg.YP, cfg.NX, cfg.PB
    PLF = cfg.PLF
    VOX2 = cfg.VOX2
    VOXH = ZS * D  # voxels per x-plane owned (512)
    NPAIR = NX // PB

    nc = bacc.Bacc("TRN2", target_bir_lowering=False, debug=False,
                   enable_asserts=False, num_devices=cfg.ncores)

    planes = nc.dram_tensor("planes", [NX * 128, PLF], dt, kind="ExternalInput").ap()
    w1 = nc.dram_tensor("w1", [C, R], dt, kind="ExternalInput").ap()
    w2 = nc.dram_tensor("w2", [R, C], dt, kind="ExternalInput").ap()
    outp = nc.dram_tensor("outp", [NX * 128, VOX2], dt, kind="ExternalOutput").ap()

    with tile.TileContext(nc) as tc, ExitStack() as ctx:
        const = ctx.enter_context(tc.tile_pool(name="const", bufs=1))
        pp = ctx.enter_context(tc.tile_pool(name="pp", bufs=3))
        ztp = ctx.enter_context(tc.tile_pool(name="ztp", bufs=2))
        ytp = ctx.enter_context(tc.tile_pool(name="ytp", bufs=2))
        oyp = ctx.enter_context(tc.tile_pool(name="oyp", bufs=3))
        m2xp = ctx.enter_context(tc.tile_pool(name="m2xp", bufs=4))
        m4xp = ctx.enter_context(tc.tile_pool(name="m4xp", bufs=6))
        pxp = ctx.enter_context(tc.tile_pool(name="pxp", bufs=3))
        hpp = ctx.enter_context(tc.tile_pool(name="hpp", bufs=2, space="PSUM"))
        y2p = ctx.enter_context(tc.tile_pool(name="y2p", bufs=2, space="PSUM"))
        hsp = ctx.enter_context(tc.tile_pool(name="hsp", bufs=3))
        sgp = ctx.enter_context(tc.tile_pool(name="sgp", bufs=3))
        prp = ctx.enter_context(tc.tile_pool(name="prp", bufs=3))

        # ---- constants
        w1sb = const.tile([128, 2 * R], dt)
        nc.sync.dma_start(
            w1sb[:].rearrange("p (h r) -> p h r", h=2),
            w1.rearrange("(h p) r -> p h r", p=128),
        )
        w2sb = const.tile([128, C], dt)
        nc.sync.dma_start(w2sb[:], w2)
        neg = const.tile([128, VOX2], dt)
        nc.gpsimd.memset(neg[:], SENT)
        w1v = w1sb[:].rearrange("p (h r) -> p h r", h=2)

        # rings hold APs (slices of pool tiles); missing entries resolve to neg
        oy_t, m2x_t, m4x_t = {}, {}, {}

        def oy_at(j):
            return oy_t.get(j, neg[:])

        def m2x_at(j):
            return m2x_t.get(j, neg[:])

        def m4x_at(j):
            return m4x_t.get(j, neg[:])

        P_t = {}  # plane tiles for the final multiply (center views)

        for i in range(NX + 3):
            if i < NX and i % PB == 0:
                # ---- load a batch of PB planes; each [128, 2h, 14z, 72y]
                P = pp.tile([128, PB * PLF], dt)
                for b in range(PB):
                    x = i + b
                    nc.sync.dma_start(
                        P[:, b * PLF:(b + 1) * PLF],
                        planes[x * 128:(x + 1) * 128, :],
                    )
                    P_t[x] = P[:, b * PLF:(b + 1) * PLF]
                # batched view: (pl h) merges into one uniform dim of 2*PB
                Pv = P[:].rearrange("p (g z y) -> p g z y", g=2 * PB, z=ZH)

                # ---- z-pass (window 7 over ZH=14 -> ZS=8), all aligned
                m2z = ztp.tile([128, 2 * PB * (ZH - 1) * YP], dt)
                m2zv = m2z[:].rearrange("p (g z y) -> p g z y", g=2 * PB, z=ZH - 1)
                nc.vector.tensor_max(m2zv, Pv[:, :, 0:ZH - 1, :], Pv[:, :, 1:ZH, :])
                m4z = ztp.tile([128, 2 * PB * (ZH - 3) * YP], dt)
                m4zv = m4z[:].rearrange("p (g z y) -> p g z y", g=2 * PB, z=ZH - 3)
                nc.vector.tensor_max(
                    m4zv, m2zv[:, :, 0:ZH - 3, :], m2zv[:, :, 2:ZH - 1, :]
                )
                z8 = ztp.tile([128, 2 * PB * ZS * YP], dt)
                z8v = z8[:].rearrange("p (g z y) -> p g z y", g=2 * PB, z=ZS)
                nc.vector.tensor_max(
                    z8v, m4zv[:, :, 0:ZS, :], m4zv[:, :, 3:3 + ZS, :]
                )

                # ---- y-pass (window 7 over YP=72 padded -> D=64)
                m2y = ytp.tile([128, 2 * PB * ZS * YP], dt)
                m2yv = m2y[:].rearrange("p (g z y) -> p g z y", g=2 * PB, z=ZS)
                nc.vector.tensor_max(
                    m2yv[:, :, :, 0:YP - 1], z8v[:, :, :, 0:YP - 1],
                    z8v[:, :, :, 1:YP],
                )
                m4y = ytp.tile([128, 2 * PB * ZS * YP], dt)
                m4yv = m4y[:].rearrange("p (g z y) -> p g z y", g=2 * PB, z=ZS)
                nc.vector.tensor_max(
                    m4yv[:, :, :, 0:YP - 3], m2yv[:, :, :, 0:YP - 3],
                    m2yv[:, :, :, 2:YP - 1],
                )
                oy = oyp.tile([128, PB * VOX2], dt)
                oyv = oy[:].rearrange("p (g z y) -> p g z y", g=2 * PB, z=ZS)
                nc.vector.tensor_max(
                    oyv, m4yv[:, :, :, 1:1 + D], m4yv[:, :, :, 4:4 + D]
                )
                for b in range(PB):
                    # per-plane compact [128, VOX2] views: g = (pl, h) so plane
                    # b's halves are g=2b, 2b+1 -> contiguous chunk of VOX2
                    oy_t[i + b] = oy[:, b * VOX2:(b + 1) * VOX2]

            # ---- x-pass (streamed, per plane); negative-index partials give
            # the left-edge clipped windows (right edge clips via aliases)
            j = i - 1
            if j == -1:
                m2x_t[j] = oy_t[0]  # max(oy[-1]=-inf, oy[0])
            elif 0 <= j < NX:
                if j + 1 < NX:
                    m2x = m2xp.tile([128, VOX2], dt)
                    nc.vector.tensor_max(m2x[:], oy_at(j), oy_at(j + 1))
                    m2x_t[j] = m2x[:]
                else:
                    m2x_t[j] = oy_t[j]
            j = i - 3
            if j == -2:
                m4x_t[j] = m2x_t[0]  # max(m2x[-2]=-inf, m2x[0])
            elif j == -1 or (0 <= j < NX - 2):
                m4x = m4xp.tile([128, VOX2], dt)
                nc.vector.tensor_max(m4x[:], m2x_at(j), m2x_at(j + 2))
                m4x_t[j] = m4x[:]
            elif 0 <= j < NX:
                m4x_t[j] = m2x_t[j]
            k = i - 3
            if 0 <= k < NX:
                if k >= 1:
                    px = pxp.tile([128, VOX2], dt)
                    nc.vector.tensor_max(px[:], m4x_at(k - 3), m4x_at(k))
                    pxa = px[:]
                else:
                    pxa = m4x_at(k)

                # ---- MLP on plane k: h = relu(W1^T @ px) on PE+ACT
                pxv = pxa.rearrange("p (h v) -> p h v", h=2)
                hp = hpp.tile([128, VOXH], f32, space="PSUM")
                for h in (0, 1):
                    nc.tensor.matmul(
                        hp[:], w1v[:, h, :], pxv[:, h, :],
                        start=(h == 0), stop=(h == 1),
                    )
                hs = hsp.tile([128, VOXH], dt)
                nc.scalar.activation(hs[:], hp[:], AF.Relu)
                # ---- y2^T = W2^T @ h: output lands [ch-part, vox]
                y2 = y2p.tile([128, 2 * VOXH], f32, space="PSUM")
                for h in (0, 1):
                    nc.tensor.matmul(
                        y2[:, h * VOXH:(h + 1) * VOXH],
                        w2sb[:, h * 128:(h + 1) * 128], hs[:],
                        start=True, stop=True,
                    )
                sg = sgp.tile([128, VOX2], dt)
                nc.scalar.activation(sg[:], y2[:], AF.Sigmoid)
                # ---- dense multiply: prod = sg * plane_center
                Pc = P_t[k].rearrange("p (h z y) -> p h z y", h=2, z=ZH)[
                    :, :, 3:3 + ZS, 4:4 + D
                ]
                prod = prp.tile([128, VOX2], dt)
                prodv = prod[:].rearrange("p (h z y) -> p h z y", h=2, z=ZS)
                nc.vector.tensor_mul(prodv, sg[:].rearrange(
                    "p (h z y) -> p h z y", h=2, z=ZS), Pc)
                nc.gpsimd.dma_start(
                    outp[k * 128:(k + 1) * 128, :], prod[:]
                )

    nc.compile()
    return nc


def host_prep(cfg: Cfg, feats, coords, W1, W2):
    """Build per-core dense slabs in device layout. Pure data marshalling."""
    D, ZS, ZH, YP, NX = cfg.D, cfg.ZS, cfg.ZH, cfg.YP, cfg.NX
    dt = _np_dt(cfg.dt)

    ix = coords[:, 0].astype(np.int64)
    iy = coords[:, 1].astype(np.int64)
    iz = coords[:, 2].astype(np.int64)

    # dense grid, padded z by 3 each side and y by 4 each side, in layout
    # [x, p(=c%128), h(=c//128), zpad, ypad]
    gridT = np.full((D, 128, 2, D + 6, YP), SENT, dtype=dt)
    fsplit = feats.astype(dt).reshape(-1, 2, 128)  # [N, h, p]
    gridT[ix, :, :, iz + 3, iy + 4] = fsplit.transpose(0, 2, 1)

    w1h = np.ascontiguousarray(W1.astype(dt))
    w2h = np.ascontiguousarray(W2.astype(dt))

    in_maps = []
    for k in range(cfg.ncores):
        slab = np.ascontiguousarray(gridT[:, :, :, 8 * k:8 * k + ZH, :])
        in_maps.append({
            "planes": slab.reshape(NX * 128, cfg.PLF),
            "w1": w1h,
            "w2": w2h,
        })
    return in_maps


def host_post(cfg: Cfg, results, coords):
    """Gather occupied voxels from the dense product planes."""
    D, ZS, NX = cfg.D, cfg.ZS, cfg.NX
    ix = coords[:, 0].astype(np.int64)
    iy = coords[:, 1].astype(np.int64)
    iz = coords[:, 2].astype(np.int64)
    out = np.empty((cfg.NPTS, C), np.float32)
    for k in range(cfg.ncores):
        sel = np.where((iz >= k * ZS) & (iz < (k + 1) * ZS))[0]
        pk = np.asarray(results[k]["outp"]).reshape(NX, 128, 2, ZS, D)
        # value for point n at channel c = h*128+p: pk[ix, p, h, iz%8, iy]
        v = pk[ix[sel], :, :, iz[sel] - k * ZS, iy[sel]]  # [n, 128, 2]
        out[sel] = v.transpose(0, 2, 1).reshape(len(sel), C).astype(np.float32)
    return out


_CACHE = {}


def _get_nc(cfg: Cfg):
    if cfg not in _CACHE:
        _CACHE[cfg] = build_nc(cfg)
    return _CACHE[cfg]


def kernel(feats, coords, W1, W2):
    from concourse.bass_utils import run_bass_kernel_spmd

    cfg = FULL
    nc = _get_nc(cfg)
    in_maps = host_prep(
        cfg,
        np.asarray(feats, np.float32),
        np.asarray(coords),
        np.asarray(W1, np.float32),
        np.asarray(W2, np.float32),
    )
    res = run_bass_kernel_spmd(nc, in_maps, core_ids=list(range(cfg.ncores)))
    return host_post(cfg, res.results, np.asarray(coords))


# revision 9
# speedup vs baseline: 1.9510x; 1.7060x over previous
"""Trainium2 Bass kernel: sparse 7x7x7 stride-1 max-pool over a 64^3 voxel grid
(MinkowskiEngine semantics) + per-point MLP (1x1 conv -> ReLU -> 1x1 conv ->
sigmoid) * feats.

Strategy (8 NeuronCores, SPMD, no collectives):
  - Shard the dense grid along z: core k owns z in [8k, 8k+8), and processes a
    14-plane z-slab (3-voxel halo each side; halo replicated on host -> no
    cross-core exchange).
  - The HOST builds the dense slab in the exact on-chip layout the kernel
    wants: per x-plane [128 part = ch%128, 2 ch-halves, 14 z, 72 y] with y
    pre-padded by 4 sentinel columns each side and empty voxels = -1e30.
    This is pure data marshalling (scatter + transpose + pad); all max/matmul
    arithmetic stays on device.
  - Device streams over the 64 x-planes: contiguous plane load, separable
    windowed max (7 = max of two 4-windows) on DVE along z then y (both in
    aligned 2x-mode layouts), then x across streamed planes; fused MLP on PE
    with the second matmul transposed (W2^T as weights) so sigmoid lands in
    [ch, vox] layout; dense multiply sg * plane-center on DVE; contiguous
    store of the dense product.
  - Host gathers the occupied voxels from the dense product planes (reverse
    marshalling) and casts to fp32.
"""

from contextlib import ExitStack
from dataclasses import dataclass

import numpy as np

C = 256
R = 128
SENT = -1.0e30


@dataclass(frozen=True)
class Cfg:
    D: int = 64           # grid extent per axis
    ZS: int = 8           # owned z-planes per core
    NPTS: int = 100000    # total points
    ncores: int = 8
    dt: str = "bfloat16"  # dtype of planes / pooling / matmul inputs
    PB: int = 2           # planes per build batch (z/y pass batching)

    @property
    def ZH(self):
        return self.ZS + 6

    @property
    def YP(self):
        return self.D + 8  # y padded to 72 (4 sentinel cols each side)

    @property
    def NX(self):
        return self.D

    @property
    def PLF(self):
        return 2 * self.ZH * self.YP  # free elems per partition per plane

    @property
    def VOX2(self):
        return 2 * self.ZS * self.D  # compact pooled free elems (2h * 8z * 64y)


FULL = Cfg()


def _np_dt(name):
    import ml_dtypes

    return {"bfloat16": ml_dtypes.bfloat16, "float16": np.float16}[name]


def build_nc(cfg: Cfg):
    """Build the (SPMD, per-core-identical) Bass program."""
    import concourse.bacc as bacc
    import concourse.tile as tile
    from concourse import mybir

    AF = mybir.ActivationFunctionType
    f32 = mybir.dt.float32
    dt = getattr(mybir.dt, cfg.dt)

    D, ZS, ZH, YP, NX, PB = cfg.D, cfg.ZS, cfg.ZH, cfg.YP, cfg.NX, cfg.PB
    PLF = cfg.PLF
    VOX2 = cfg.VOX2
    VOXH = ZS * D  # voxels per x-plane owned (512)
    NPAIR = NX // PB

    nc = bacc.Bacc("TRN2", target_bir_lowering=False, debug=False,
                   enable_asserts=False, num_devices=cfg.ncores)

    planes = nc.dram_tensor("planes", [NX * 128, PLF], dt, kind="ExternalInput").ap()
    w1 = nc.dram_tensor("w1", [C, R], dt, kind="ExternalInput").ap()
    w2 = nc.dram_tensor("w2", [R, C], dt, kind="ExternalInput").ap()
    outp = nc.dram_tensor("outp", [NX * 128, VOX2], dt, kind="ExternalOutput").ap()

    with tile.TileContext(nc) as tc, ExitStack() as ctx:
        const = ctx.enter_context(tc.tile_pool(name="const", bufs=1))
        pp = ctx.enter_context(tc.tile_pool(name="pp", bufs=6))
        ztp = ctx.enter_context(tc.tile_pool(name="ztp", bufs=2))
        ytp = ctx.enter_context(tc.tile_pool(name="ytp", bufs=2))
        oyp = ctx.enter_context(tc.tile_pool(name="oyp", bufs=3))
        m2xp = ctx.enter_context(tc.tile_pool(name="m2xp", bufs=4))
        m4xp = ctx.enter_context(tc.tile_pool(name="m4xp", bufs=6))
        pxp = ctx.enter_context(tc.tile_pool(name="pxp", bufs=3))
        hpp = ctx.enter_context(tc.tile_pool(name="hpp", bufs=2, space="PSUM"))
        y2p = ctx.enter_context(tc.tile_pool(name="y2p", bufs=2, space="PSUM"))
        hsp = ctx.enter_context(tc.tile_pool(name="hsp", bufs=3))
        sgp = ctx.enter_context(tc.tile_pool(name="sgp", bufs=5))
        prp = ctx.enter_context(tc.tile_pool(name="prp", bufs=3))

        # ---- constants
        w1sb = const.tile([128, 2 * R], dt)
        nc.sync.dma_start(
            w1sb[:].rearrange("p (h r) -> p h r", h=2),
            w1.rearrange("(h p) r -> p h r", p=128),
        )
        w2sb = const.tile([128, C], dt)
        nc.sync.dma_start(w2sb[:], w2)
        neg = const.tile([128, VOX2], dt)
        nc.gpsimd.memset(neg[:], SENT)
        w1v = w1sb[:].rearrange("p (h r) -> p h r", h=2)

        # rings hold APs (slices of pool tiles); missing entries resolve to neg
        oy_t, m2x_t, m4x_t = {}, {}, {}

        def oy_at(j):
            return oy_t.get(j, neg[:])

        def m2x_at(j):
            return m2x_t.get(j, neg[:])

        def m4x_at(j):
            return m4x_t.get(j, neg[:])

        P_t = {}  # plane tiles for the final multiply (center views)
        Pp_t = {}  # prefetched pair tiles
        sg_t = {}  # sigmoid tiles awaiting the delayed multiply

        def load_pair(pi):
            P = pp.tile([128, PB * PLF], dt)
            for b in range(PB):
                x = pi * PB + b
                nc.sync.dma_start(
                    P[:, b * PLF:(b + 1) * PLF],
                    planes[x * 128:(x + 1) * 128, :],
                )
                P_t[x] = P[:, b * PLF:(b + 1) * PLF]
            Pp_t[pi] = P

        load_pair(0)
        load_pair(1)

        for i in range(NX + 5):
            if i < NX and i % PB == 0:
                pi = i // PB
                if pi + 2 < NX // PB:
                    load_pair(pi + 2)
                P = Pp_t.pop(pi)
                # batched view: (pl h) merges into one uniform dim of 2*PB
                Pv = P[:].rearrange("p (g z y) -> p g z y", g=2 * PB, z=ZH)

                # ---- z-pass (window 7 over ZH=14 -> ZS=8), all aligned
                m2z = ztp.tile([128, 2 * PB * (ZH - 1) * YP], dt)
                m2zv = m2z[:].rearrange("p (g z y) -> p g z y", g=2 * PB, z=ZH - 1)
                nc.vector.tensor_max(m2zv, Pv[:, :, 0:ZH - 1, :], Pv[:, :, 1:ZH, :])
                m4z = ztp.tile([128, 2 * PB * (ZH - 3) * YP], dt)
                m4zv = m4z[:].rearrange("p (g z y) -> p g z y", g=2 * PB, z=ZH - 3)
                nc.vector.tensor_max(
                    m4zv, m2zv[:, :, 0:ZH - 3, :], m2zv[:, :, 2:ZH - 1, :]
                )
                z8 = ztp.tile([128, 2 * PB * ZS * YP], dt)
                z8v = z8[:].rearrange("p (g z y) -> p g z y", g=2 * PB, z=ZS)
                nc.vector.tensor_max(
                    z8v, m4zv[:, :, 0:ZS, :], m4zv[:, :, 3:3 + ZS, :]
                )

                # ---- y-pass (window 7 over YP=72 padded -> D=64)
                m2y = ytp.tile([128, 2 * PB * ZS * YP], dt)
                m2yv = m2y[:].rearrange("p (g z y) -> p g z y", g=2 * PB, z=ZS)
                nc.vector.tensor_max(
                    m2yv[:, :, :, 0:YP - 1], z8v[:, :, :, 0:YP - 1],
                    z8v[:, :, :, 1:YP],
                )
                m4y = ytp.tile([128, 2 * PB * ZS * YP], dt)
                m4yv = m4y[:].rearrange("p (g z y) -> p g z y", g=2 * PB, z=ZS)
                nc.vector.tensor_max(
                    m4yv[:, :, :, 0:YP - 3], m2yv[:, :, :, 0:YP - 3],
                    m2yv[:, :, :, 2:YP - 1],
                )
                oy = oyp.tile([128, PB * VOX2], dt)
                oyv = oy[:].rearrange("p (g z y) -> p g z y", g=2 * PB, z=ZS)
                nc.vector.tensor_max(
                    oyv, m4yv[:, :, :, 1:1 + D], m4yv[:, :, :, 4:4 + D]
                )
                for b in range(PB):
                    # per-plane compact [128, VOX2] views: g = (pl, h) so plane
                    # b's halves are g=2b, 2b+1 -> contiguous chunk of VOX2
                    oy_t[i + b] = oy[:, b * VOX2:(b + 1) * VOX2]

            # ---- x-pass (streamed, per plane); negative-index partials give
            # the left-edge clipped windows (right edge clips via aliases)
            j = i - 1
            if j == -1:
                m2x_t[j] = oy_t[0]  # max(oy[-1]=-inf, oy[0])
            elif 0 <= j < NX:
                if j + 1 < NX:
                    m2x = m2xp.tile([128, VOX2], dt)
                    nc.vector.tensor_max(m2x[:], oy_at(j), oy_at(j + 1))
                    m2x_t[j] = m2x[:]
                else:
                    m2x_t[j] = oy_t[j]
            j = i - 3
            if j == -2:
                m4x_t[j] = m2x_t[0]  # max(m2x[-2]=-inf, m2x[0])
            elif j == -1 or (0 <= j < NX - 2):
                m4x = m4xp.tile([128, VOX2], dt)
                nc.vector.tensor_max(m4x[:], m2x_at(j), m2x_at(j + 2))
                m4x_t[j] = m4x[:]
            elif 0 <= j < NX:
                m4x_t[j] = m2x_t[j]
            k = i - 3
            if 0 <= k < NX:
                if k >= 1:
                    px = pxp.tile([128, VOX2], dt)
                    nc.vector.tensor_max(px[:], m4x_at(k - 3), m4x_at(k))
                    pxa = px[:]
                else:
                    pxa = m4x_at(k)

                # ---- MLP on plane k: h = relu(W1^T @ px) on PE+ACT
                pxv = pxa.rearrange("p (h v) -> p h v", h=2)
                hp = hpp.tile([128, VOXH], f32, space="PSUM")
                for h in (0, 1):
                    nc.tensor.matmul(
                        hp[:], w1v[:, h, :], pxv[:, h, :],
                        start=(h == 0), stop=(h == 1),
                    )
                hs = hsp.tile([128, VOXH], dt)
                nc.scalar.activation(hs[:], hp[:], AF.Relu)
                # ---- y2^T = W2^T @ h: output lands [ch-part, vox]
                y2 = y2p.tile([128, 2 * VOXH], f32, space="PSUM")
                for h in (0, 1):
                    nc.tensor.matmul(
                        y2[:, h * VOXH:(h + 1) * VOXH],
                        w2sb[:, h * 128:(h + 1) * 128], hs[:],
                        start=True, stop=True,
                    )
                sg = sgp.tile([128, VOX2], dt)
                nc.scalar.activation(sg[:], y2[:], AF.Sigmoid)
                sg_t[k] = sg

            # ---- dense multiply (delayed 2 steps so the PE/ACT round trip
            # never blocks the in-order DVE queue): prod = sg * plane_center
            k2 = i - 5
            if 0 <= k2 < NX:
                sg = sg_t.pop(k2)
                Pc = P_t.pop(k2).rearrange("p (h z y) -> p h z y", h=2, z=ZH)[
                    :, :, 3:3 + ZS, 4:4 + D
                ]
                prod = prp.tile([128, VOX2], dt)
                prodv = prod[:].rearrange("p (h z y) -> p h z y", h=2, z=ZS)
                nc.vector.tensor_mul(prodv, sg[:].rearrange(
                    "p (h z y) -> p h z y", h=2, z=ZS), Pc)
                nc.gpsimd.dma_start(
                    outp[k2 * 128:(k2 + 1) * 128, :], prod[:]
                )

    nc.compile()
    return nc


def host_prep(cfg: Cfg, feats, coords, W1, W2):
    """Build per-core dense slabs in device layout. Pure data marshalling."""
    D, ZS, ZH, YP, NX = cfg.D, cfg.ZS, cfg.ZH, cfg.YP, cfg.NX
    dt = _np_dt(cfg.dt)

    ix = coords[:, 0].astype(np.int64)
    iy = coords[:, 1].astype(np.int64)
    iz = coords[:, 2].astype(np.int64)

    # dense grid, padded z by 3 each side and y by 4 each side, in layout
    # [x, p(=c%128), h(=c//128), zpad, ypad]
    gridT = np.full((D, 128, 2, D + 6, YP), SENT, dtype=dt)
    fsplit = feats.astype(dt).reshape(-1, 2, 128)  # [N, h, p]
    gridT[ix, :, :, iz + 3, iy + 4] = fsplit.transpose(0, 2, 1)

    w1h = np.ascontiguousarray(W1.astype(dt))
    w2h = np.ascontiguousarray(W2.astype(dt))

    in_maps = []
    for k in range(cfg.ncores):
        slab = np.ascontiguousarray(gridT[:, :, :, 8 * k:8 * k + ZH, :])
        in_maps.append({
            "planes": slab.reshape(NX * 128, cfg.PLF),
            "w1": w1h,
            "w2": w2h,
        })
    return in_maps


def host_post(cfg: Cfg, results, coords):
    """Gather occupied voxels from the dense product planes."""
    D, ZS, NX = cfg.D, cfg.ZS, cfg.NX
    ix = coords[:, 0].astype(np.int64)
    iy = coords[:, 1].astype(np.int64)
    iz = coords[:, 2].astype(np.int64)
    out = np.empty((cfg.NPTS, C), np.float32)
    for k in range(cfg.ncores):
        sel = np.where((iz >= k * ZS) & (iz < (k + 1) * ZS))[0]
        pk = np.asarray(results[k]["outp"]).reshape(NX, 128, 2, ZS, D)
        # value for point n at channel c = h*128+p: pk[ix, p, h, iz%8, iy]
        v = pk[ix[sel], :, :, iz[sel] - k * ZS, iy[sel]]  # [n, 128, 2]
        out[sel] = v.transpose(0, 2, 1).reshape(len(sel), C).astype(np.float32)
    return out


_CACHE = {}


def _get_nc(cfg: Cfg):
    if cfg not in _CACHE:
        _CACHE[cfg] = build_nc(cfg)
    return _CACHE[cfg]


def kernel(feats, coords, W1, W2):
    from concourse.bass_utils import run_bass_kernel_spmd

    cfg = FULL
    nc = _get_nc(cfg)
    in_maps = host_prep(
        cfg,
        np.asarray(feats, np.float32),
        np.asarray(coords),
        np.asarray(W1, np.float32),
        np.asarray(W2, np.float32),
    )
    res = run_bass_kernel_spmd(nc, in_maps, core_ids=list(range(cfg.ncores)))
    return host_post(cfg, res.results, np.asarray(coords))
